# revision 7
# baseline (speedup 1.0000x reference)
"""Trainium2 Bass kernel for nn_DiversityMetric (batched NND diversity metric).

Math (per batch b, X = pred_poses[b] in R^{N x D}, N=2048, D=128):
    sq_dist[i,j] = ||xi||^2 + ||xj||^2 - 2 <xi,xj>, diag = inf
    nnd[i]       = sqrt(min_{j != i} sq_dist[i,j])
    out          = [mean(nnd), std(nnd, ddof=1), cv]   over all B*N points

Device strategy (8 cores, 2 batches/core):
    - Host pre-transposes each batch to XT [D=128, N=2048] so the contraction
      dim D sits on SBUF partitions; no on-device transpose needed.
    - PSUM block [128, 512] accumulates   g_ij - 0.5*sqn_j   via two fp32r
      matmuls: MM1 (lhsT = XT m-block, rhs = XT j-block) and MM2
      (lhsT = -0.5*ones, rhs = sq = XT^2).  Diagonal blocks get a third tiny
      bf16 matmul adding -1e6*I.
    - Then   nnd_i = sqrt(relu(sqn_i - 2 * max_j (g_ij - 0.5*sqn_j)))
      so the epilogue is a plain reduce_max streamed from PSUM on DVE.
    - sqn_i columns come from tiny N=1 matmuls (lhsT = sq m-block, rhs = ones).
    - Host computes the final mean/std/cv from the gathered 16x2048 NND matrix
      (O(B*N) scalar tail).
"""

import numpy as np
from contextlib import ExitStack

import ml_dtypes

import concourse.bass as bass
import concourse.bacc as bacc
import concourse.mybir as mybir
import concourse.tile as tile
from concourse.bass_utils import run_bass_kernel_spmd

F32 = mybir.dt.float32
F32R = mybir.dt.float32r
BF16 = mybir.dt.bfloat16

B, N, D = 16, 2048, 128
NCORES = 8
BPC = B // NCORES          # batches per core
P = 128                    # partitions
MBLK = N // P              # 16 row blocks per batch
HALF = 1024                # columns handled per psum tile
NBANK = HALF // 512        # matmuls per psum tile
NEGBIG = -1.0e6

_CACHE = {}


def _r(ap):
    """bitcast an f32 AP to float32r for full-rate PE matmul."""
    return ap.bitcast(F32R)


def build_kernel():
    nc = bacc.Bacc("TRN2", target_bir_lowering=False, debug=False)

    xt_d = nc.dram_tensor("xt", [BPC, P, N], F32R, kind="ExternalInput")
    neghalf_d = nc.dram_tensor("neghalf", [P, P], F32R, kind="ExternalInput")
    ones_d = nc.dram_tensor("onescol", [P, 2], F32R, kind="ExternalInput")
    identneg_d = nc.dram_tensor("identneg", [P, P], BF16, kind="ExternalInput")
    ident_d = nc.dram_tensor("ident", [P, P], BF16, kind="ExternalInput")
    nnd_d = nc.dram_tensor("nnd", [P, BPC * MBLK], F32, kind="ExternalOutput")

    with tile.TileContext(nc) as tc, ExitStack() as ctx:
        const = ctx.enter_context(tc.tile_pool(name="const", bufs=1))
        xpool = ctx.enter_context(tc.tile_pool(name="x", bufs=1))
        spool = ctx.enter_context(tc.tile_pool(name="s", bufs=1))
        small = ctx.enter_context(tc.tile_pool(name="small", bufs=1))
        psum = ctx.enter_context(tc.tile_pool(name="psum", bufs=3, space="PSUM"))
        psq = ctx.enter_context(tc.tile_pool(name="psq", bufs=1, space="PSUM"))

        neghalf = const.tile([P, P], F32R)
        nc.sync.dma_start(neghalf[:], neghalf_d.ap())
        onescol = const.tile([P, 2], F32R)
        nc.sync.dma_start(onescol[:], ones_d.ap())
        identneg = const.tile([P, P], BF16)
        nc.sync.dma_start(identneg[:], identneg_d.ap())
        ident = const.tile([P, P], BF16)
        nc.sync.dma_start(ident[:], ident_d.ap())

        # per-batch chunked loads: xt[b] is [128, 2048]; 2 chunks of 1024 each
        NCHUNK = N // HALF
        xts = {}
        sqs = {}
        for b in range(BPC):
            for c in range(NCHUNK):
                xtile = xpool.tile([P, HALF], F32R, tag=f"xt_{b}_{c}")
                nc.sync.dma_start(
                    xtile[:], xt_d.ap()[b, :, c * HALF:(c + 1) * HALF]
                )
                xts[(b, c)] = xtile
                stile = spool.tile([P, HALF], F32R, tag=f"sq_{b}_{c}")
                nc.scalar.square(stile[:], xtile[:])
                sqs[(b, c)] = stile

        def xcol(b, j0, w):
            """AP for xt[b][:, j0:j0+w] (within one chunk)."""
            c = j0 // HALF
            off = j0 - c * HALF
            assert off + w <= HALF
            return xts[(b, c)][:, off:off + w]

        def scol(b, j0, w):
            c = j0 // HALF
            off = j0 - c * HALF
            assert off + w <= HALF
            return sqs[(b, c)][:, off:off + w]

        # sqn_i columns: tiny matmuls  psum_sqn[:, 2*col] = sum_d sq[d, i-block]
        # (fp32r ISA needs even innermost counts, so each matmul emits 2
        # identical columns and the copy-out reads every other one)
        psum_sqn = psq.tile([P, 2 * BPC * MBLK], F32)
        for b in range(BPC):
            for m in range(MBLK):
                col = b * MBLK + m
                nc.tensor.matmul(
                    psum_sqn[:, 2 * col:2 * col + 2],
                    scol(b, m * P, P),
                    onescol[:],
                    start=True, stop=True,
                )
        sqn_cols = small.tile([P, BPC * MBLK], F32)
        nc.vector.tensor_copy(
            sqn_cols[:],
            psum_sqn[:].rearrange("p (c t) -> p c t", t=2)[:, :, 0:1],
        )

        # main loop: for each (batch, row-block, half) compute
        # psum = g - 0.5*sqn_j (+ diag mask), then reduce_max over j.
        rmax2 = small.tile([P, BPC * MBLK, N // HALF], F32)
        for b in range(BPC):
            for m in range(MBLK):
                col = b * MBLK + m
                lhs_x = xcol(b, m * P, P)
                for h in range(N // HALF):
                    ph = psum.tile([P, HALF], F32, tag="ph")
                    for k in range(NBANK):
                        j0 = h * HALF + k * 512
                        diag_here = (m * P) // 512 == j0 // 512
                        nc.tensor.matmul(
                            ph[:, k * 512:(k + 1) * 512],
                            lhs_x,
                            xcol(b, j0, 512),
                            start=True, stop=False,
                        )
                        nc.tensor.matmul(
                            ph[:, k * 512:(k + 1) * 512],
                            neghalf[:],
                            scol(b, j0, 512),
                            start=False, stop=not diag_here,
                        )
                        if diag_here:
                            off = m * P - h * HALF
                            nc.tensor.matmul(
                                ph[:, off:off + P],
                                identneg[:],
                                ident[:],
                                start=False, stop=True,
                            )
                    nc.vector.tensor_reduce(
                        rmax2[:, col, h:h + 1],
                        ph[:],
                        axis=mybir.AxisListType.X,
                        op=mybir.AluOpType.max,
                    )

        # nnd = sqrt(relu(sqn_i + (-2)*max))
        rmax = small.tile([P, BPC * MBLK], F32)
        nc.vector.tensor_reduce(
            rmax[:], rmax2[:], axis=mybir.AxisListType.X, op=mybir.AluOpType.max
        )
        nnd2 = small.tile([P, BPC * MBLK], F32)
        nc.vector.scalar_tensor_tensor(
            nnd2[:], rmax[:], -2.0, sqn_cols[:],
            op0=mybir.AluOpType.mult, op1=mybir.AluOpType.add,
        )
        nc.vector.tensor_scalar_max(nnd2[:], nnd2[:], 0.0)
        nnd_sb = small.tile([P, BPC * MBLK], F32)
        nc.scalar.sqrt(nnd_sb[:], nnd2[:])
        nc.sync.dma_start(nnd_d.ap()[:, :], nnd_sb[:])

    nc.compile()
    return nc


def _consts():
    neghalf = np.full((P, P), -0.5, dtype=np.float32)
    onescol = np.ones((P, 2), dtype=np.float32)
    identneg = (NEGBIG * np.eye(P)).astype(ml_dtypes.bfloat16)
    ident = np.eye(P, dtype=np.float32).astype(ml_dtypes.bfloat16)
    return neghalf, onescol, identneg, ident


def kernel(pred_poses: np.ndarray) -> np.ndarray:
    pred_poses = np.ascontiguousarray(np.asarray(pred_poses, dtype=np.float32))
    assert pred_poses.shape == (B, N, D)

    if "nc" not in _CACHE:
        _CACHE["nc"] = build_kernel()
    nc = _CACHE["nc"]

    neghalf, onescol, identneg, ident = _consts()
    in_maps = []
    for c in range(NCORES):
        xb = pred_poses[c * BPC:(c + 1) * BPC]          # [BPC, N, D]
        xt = np.ascontiguousarray(xb.transpose(0, 2, 1))  # [BPC, D, N]
        in_maps.append({
            "xt": xt,
            "neghalf": neghalf,
            "onescol": onescol,
            "identneg": identneg,
            "ident": ident,
        })

    res = run_bass_kernel_spmd(nc, in_maps, list(range(NCORES)))

    nnd = np.zeros((B, N), dtype=np.float64)
    for c in range(NCORES):
        t = np.asarray(res.results[c]["nnd"])           # [128, BPC*MBLK]
        for bl in range(BPC):
            sub = t[:, bl * MBLK:(bl + 1) * MBLK]       # [128, 16] (p, m)
            nnd[c * BPC + bl] = sub.T.reshape(N)        # index m*128+p

    mean = nnd.mean()
    std = nnd.std(ddof=1)
    eps = 1e-8
    cv = std / max(mean, eps) if mean > eps else 0.0
    return np.stack([mean, std, cv]).astype(np.float32)


# revision 8
# speedup vs baseline: 1.1144x; 1.1144x over previous
"""Trainium2 Bass kernel for nn_DiversityMetric (batched NND diversity metric).

Math (per batch b, X = pred_poses[b] in R^{N x D}, N=2048, D=128):
    sq_dist[i,j] = ||xi||^2 + ||xj||^2 - 2 <xi,xj>, diag = inf
    nnd[i]       = sqrt(min_{j != i} sq_dist[i,j])
    out          = [mean(nnd), std(nnd, ddof=1), cv]   over all B*N points

Device strategy (8 cores, 2 batches/core):
    - Host pre-transposes each batch to XT [D=128, N=2048] (bf16) so the
      contraction dim D sits on SBUF partitions; no on-device transpose.
    - PSUM block [128, 512] accumulates   g_ij - 0.5*sqn_j   via two bf16
      matmuls: MM1 (lhsT = XT m-block, rhs = XT j-block) and MM2
      (lhsT = -0.5 fill, rhs = sq = XT^2).  Diagonal blocks get a third
      tiny bf16 matmul adding -1e6*I.
    - Then   nnd_i = sqrt(relu(sqn_i - 2 * max_j (g_ij - 0.5*sqn_j)))
      so the epilogue is a plain reduce_max streamed from PSUM on DVE.
    - sqn_i columns come from tiny N=2 matmuls (lhsT = sq m-block, rhs = ones).
    - Host computes the final mean/std/cv from the gathered 16x2048 NND matrix
      (O(B*N) scalar tail).
bf16 keeps nnd error ~5e-4 relative: the maxed values g-0.5*sqn_j sit near
small magnitudes (|v|~5..40) where bf16 absolute error is tiny.
"""

import numpy as np
from contextlib import ExitStack

import ml_dtypes

import concourse.bass as bass
import concourse.bacc as bacc
import concourse.mybir as mybir
import concourse.tile as tile
from concourse.bass_utils import run_bass_kernel_spmd

F32 = mybir.dt.float32
BF16 = mybir.dt.bfloat16

B, N, D = 16, 2048, 128
NCORES = 8
BPC = B // NCORES          # batches per core
P = 128                    # partitions
MBLK = N // P              # 16 row blocks per batch
HALF = 1024                # columns handled per psum tile
MMW = 512                  # matmul moving width (1 PSUM bank)
NEGBIG = -1.0e6

_CACHE = {}


def build_kernel():
    nc = bacc.Bacc("TRN2", target_bir_lowering=False, debug=False)

    xt_d = nc.dram_tensor("xt", [BPC, P, N], BF16, kind="ExternalInput")
    neghalf_d = nc.dram_tensor("neghalf", [P, P], BF16, kind="ExternalInput")
    ones_d = nc.dram_tensor("onescol", [P, 2], BF16, kind="ExternalInput")
    identneg_d = nc.dram_tensor("identneg", [P, P], BF16, kind="ExternalInput")
    ident_d = nc.dram_tensor("ident", [P, P], BF16, kind="ExternalInput")
    nnd_d = nc.dram_tensor("nnd", [P, BPC * MBLK], F32, kind="ExternalOutput")

    with tile.TileContext(nc) as tc, ExitStack() as ctx:
        const = ctx.enter_context(tc.tile_pool(name="const", bufs=1))
        xpool = ctx.enter_context(tc.tile_pool(name="x", bufs=1))
        spool = ctx.enter_context(tc.tile_pool(name="s", bufs=1))
        small = ctx.enter_context(tc.tile_pool(name="small", bufs=1))
        psum = ctx.enter_context(tc.tile_pool(name="psum", bufs=3, space="PSUM"))
        psq = ctx.enter_context(tc.tile_pool(name="psq", bufs=1, space="PSUM"))

        neghalf = const.tile([P, P], BF16)
        nc.sync.dma_start(neghalf[:], neghalf_d.ap())
        onescol = const.tile([P, 2], BF16)
        nc.sync.dma_start(onescol[:], ones_d.ap())
        identneg = const.tile([P, P], BF16)
        nc.sync.dma_start(identneg[:], identneg_d.ap())
        ident = const.tile([P, P], BF16)
        nc.sync.dma_start(ident[:], ident_d.ap())

        # per-batch chunked loads: xt[b] is [128, 2048]; chunks of 1024
        NCHUNK = N // HALF
        xts = {}
        sqs = {}
        for b in range(BPC):
            for c in range(NCHUNK):
                xtile = xpool.tile([P, HALF], BF16, tag=f"xt_{b}_{c}")
                nc.sync.dma_start(
                    xtile[:], xt_d.ap()[b, :, c * HALF:(c + 1) * HALF]
                )
                xts[(b, c)] = xtile
                stile = spool.tile([P, HALF], BF16, tag=f"sq_{b}_{c}")
                nc.scalar.square(stile[:], xtile[:])
                sqs[(b, c)] = stile

        def xcol(b, j0, w):
            c = j0 // HALF
            off = j0 - c * HALF
            assert off + w <= HALF
            return xts[(b, c)][:, off:off + w]

        def scol(b, j0, w):
            c = j0 // HALF
            off = j0 - c * HALF
            assert off + w <= HALF
            return sqs[(b, c)][:, off:off + w]

        # sqn_i columns: tiny matmuls  psum_sqn[:, 2c:2c+2] = sum_d sq[d, i-blk]
        psum_sqn = psq.tile([P, 2 * BPC * MBLK], F32)
        for b in range(BPC):
            for m in range(MBLK):
                col = b * MBLK + m
                nc.tensor.matmul(
                    psum_sqn[:, 2 * col:2 * col + 2],
                    scol(b, m * P, P),
                    onescol[:],
                    start=True, stop=True,
                )
        sqn_cols = small.tile([P, BPC * MBLK], F32)
        nc.vector.tensor_copy(
            sqn_cols[:],
            psum_sqn[:].rearrange("p (c t) -> p c t", t=2)[:, :, 0:1],
        )

        # main loop: for each (batch, row-block, half) compute
        # psum = g - 0.5*sqn_j (+ diag mask), then reduce_max over j.
        rmax2 = small.tile([P, BPC * MBLK, N // HALF], F32)
        for b in range(BPC):
            for m in range(MBLK):
                col = b * MBLK + m
                lhs_x = xcol(b, m * P, P)
                for h in range(N // HALF):
                    ph = psum.tile([P, HALF], F32, tag="ph")
                    for k in range(HALF // MMW):
                        j0 = h * HALF + k * MMW
                        diag_here = (m * P) // MMW == j0 // MMW
                        nc.tensor.matmul(
                            ph[:, k * MMW:(k + 1) * MMW],
                            lhs_x,
                            xcol(b, j0, MMW),
                            start=True, stop=False,
                        )
                        nc.tensor.matmul(
                            ph[:, k * MMW:(k + 1) * MMW],
                            neghalf[:],
                            scol(b, j0, MMW),
                            start=False, stop=not diag_here,
                        )
                        if diag_here:
                            off = m * P - h * HALF
                            nc.tensor.matmul(
                                ph[:, off:off + P],
                                identneg[:],
                                ident[:],
                                start=False, stop=True,
                            )
                    nc.vector.tensor_reduce(
                        rmax2[:, col, h:h + 1],
                        ph[:],
                        axis=mybir.AxisListType.X,
                        op=mybir.AluOpType.max,
                    )

        # nnd = sqrt(relu(sqn_i + (-2)*max))
        rmax = small.tile([P, BPC * MBLK], F32)
        nc.vector.tensor_reduce(
            rmax[:], rmax2[:], axis=mybir.AxisListType.X, op=mybir.AluOpType.max
        )
        nnd2 = small.tile([P, BPC * MBLK], F32)
        nc.vector.scalar_tensor_tensor(
            nnd2[:], rmax[:], -2.0, sqn_cols[:],
            op0=mybir.AluOpType.mult, op1=mybir.AluOpType.add,
        )
        nc.vector.tensor_scalar_max(nnd2[:], nnd2[:], 0.0)
        nnd_sb = small.tile([P, BPC * MBLK], F32)
        nc.scalar.sqrt(nnd_sb[:], nnd2[:])
        nc.sync.dma_start(nnd_d.ap()[:, :], nnd_sb[:])

    nc.compile()
    return nc


def _consts():
    neghalf = np.full((P, P), -0.5, dtype=ml_dtypes.bfloat16)
    onescol = np.ones((P, 2), dtype=ml_dtypes.bfloat16)
    identneg = (NEGBIG * np.eye(P)).astype(ml_dtypes.bfloat16)
    ident = np.eye(P, dtype=np.float32).astype(ml_dtypes.bfloat16)
    return neghalf, onescol, identneg, ident


def kernel(pred_poses: np.ndarray) -> np.ndarray:
    pred_poses = np.ascontiguousarray(np.asarray(pred_poses, dtype=np.float32))
    assert pred_poses.shape == (B, N, D)

    if "nc" not in _CACHE:
        _CACHE["nc"] = build_kernel()
    nc = _CACHE["nc"]

    neghalf, onescol, identneg, ident = _consts()
    in_maps = []
    for c in range(NCORES):
        xb = pred_poses[c * BPC:(c + 1) * BPC]          # [BPC, N, D]
        xt = np.ascontiguousarray(
            xb.transpose(0, 2, 1)).astype(ml_dtypes.bfloat16)  # [BPC, D, N]
        in_maps.append({
            "xt": xt,
            "neghalf": neghalf,
            "onescol": onescol,
            "identneg": identneg,
            "ident": ident,
        })

    res = run_bass_kernel_spmd(nc, in_maps, list(range(NCORES)))

    nnd = np.zeros((B, N), dtype=np.float64)
    for c in range(NCORES):
        t = np.asarray(res.results[c]["nnd"])           # [128, BPC*MBLK]
        for bl in range(BPC):
            sub = t[:, bl * MBLK:(bl + 1) * MBLK]       # [128, 16] (p, m)
            nnd[c * BPC + bl] = sub.T.reshape(N)        # index m*128+p

    mean = nnd.mean()
    std = nnd.std(ddof=1)
    eps = 1e-8
    cv = std / max(mean, eps) if mean > eps else 0.0
    return np.stack([mean, std, cv]).astype(np.float32)


# revision 14
# speedup vs baseline: 1.2250x; 1.0992x over previous
"""Trainium2 Bass kernel for nn_DiversityMetric (batched NND diversity metric).

Math (per batch b, X = pred_poses[b] in R^{N x D}, N=2048, D=128):
    sq_dist[i,j] = ||xi||^2 + ||xj||^2 - 2 <xi,xj>, diag = inf
    nnd[i]       = sqrt(min_{j != i} sq_dist[i,j])
    out          = [mean(nnd), std(nnd, ddof=1), cv]   over all B*N points

Device strategy (8 cores, 2 batches/core), all-bf16 data path:
    - Host pre-transposes each batch to XT [D=128, N=2048] (bf16).
    - PSUM row-half [128, 1024] accumulates  g_ij - 0.5*sqn_j  via two bf16
      matmuls (lhsT = XT m-block / -0.5 fill); diagonal gets a tiny bf16
      matmul adding -1e6*I.   nnd_i = sqrt(relu(sqn_i - 2*max_j(...))).
    - The max-reduction (the bottleneck: every PSUM element must stream
      through a 1-elem/cycle port) is split across three engine paths:
        path A: DVE tensor_reduce straight from PSUM        (1.04 ns/elem)
        path B: ACT copy PSUM->SBUF bf16, DVE TT-max @2x, DVE reduce
        path C: ACT copy PSUM->SBUF bf16, GPSIMD TT-max folds, small DVE
      so DVE, ACT and GPSIMD all chew on it concurrently.
    - sqn_i columns from tiny N=2 matmuls (lhsT = sq m-block, rhs = ones).
    - Host computes final mean/std/cv from the gathered 16x2048 NND matrix.
"""

import numpy as np
from contextlib import ExitStack

import ml_dtypes

import concourse.bass as bass
import concourse.bacc as bacc
import concourse.mybir as mybir
import concourse.tile as tile
from concourse.bass_utils import run_bass_kernel_spmd

F32 = mybir.dt.float32
BF16 = mybir.dt.bfloat16

B, N, D = 16, 2048, 128
NCORES = 8
BPC = B // NCORES          # batches per core
P = 128                    # partitions
MBLK = N // P              # 16 row blocks per batch
HALF = 1024                # columns per psum tile
MMW = 512                  # matmul moving width
NEGBIG = -1.0e6

# epilogue path per row index (b*MBLK+m):
# 'A' = direct DVE reduce from PSUM, 'B' = ACT copy to bf16 + DVE fold+reduce
PATTERN = ['A' if i % 6 == 0 else 'B' for i in range(BPC * MBLK)]

_CACHE = {}


def build_kernel():
    nc = bacc.Bacc("TRN2", target_bir_lowering=False, debug=False)

    xt_d = nc.dram_tensor("xt", [BPC, P, N], BF16, kind="ExternalInput")
    neghalf_d = nc.dram_tensor("neghalf", [P, P], BF16, kind="ExternalInput")
    ones_d = nc.dram_tensor("onescol", [P, 2], BF16, kind="ExternalInput")
    identneg_d = nc.dram_tensor("identneg", [P, P], BF16, kind="ExternalInput")
    ident_d = nc.dram_tensor("ident", [P, P], BF16, kind="ExternalInput")
    nnd_d = nc.dram_tensor("nnd", [P, BPC * MBLK], F32, kind="ExternalOutput")

    with tile.TileContext(nc) as tc, ExitStack() as ctx:
        const = ctx.enter_context(tc.tile_pool(name="const", bufs=1))
        xpool = ctx.enter_context(tc.tile_pool(name="x", bufs=1))
        spool = ctx.enter_context(tc.tile_pool(name="s", bufs=1))
        small = ctx.enter_context(tc.tile_pool(name="small", bufs=1))
        cpool = ctx.enter_context(tc.tile_pool(name="cp", bufs=4))
        fpool = ctx.enter_context(tc.tile_pool(name="fold", bufs=3))
        psum = ctx.enter_context(tc.tile_pool(name="psum", bufs=3, space="PSUM"))
        psq = ctx.enter_context(tc.tile_pool(name="psq", bufs=1, space="PSUM"))

        neghalf = const.tile([P, P], BF16)
        nc.sync.dma_start(neghalf[:], neghalf_d.ap())
        onescol = const.tile([P, 2], BF16)
        nc.sync.dma_start(onescol[:], ones_d.ap())
        identneg = const.tile([P, P], BF16)
        nc.sync.dma_start(identneg[:], identneg_d.ap())
        ident = const.tile([P, P], BF16)
        nc.sync.dma_start(ident[:], ident_d.ap())

        # per-batch chunked loads: xt[b] is [128, 2048]; chunks of 1024
        NCHUNK = N // HALF
        xts = {}
        sqs = {}
        for b in range(BPC):
            for c in range(NCHUNK):
                xtile = xpool.tile([P, HALF], BF16, tag=f"xt_{b}_{c}")
                nc.sync.dma_start(
                    xtile[:], xt_d.ap()[b, :, c * HALF:(c + 1) * HALF]
                )
                xts[(b, c)] = xtile
                stile = spool.tile([P, HALF], BF16, tag=f"sq_{b}_{c}")
                nc.scalar.square(stile[:], xtile[:])
                sqs[(b, c)] = stile

        def xcol(b, j0, w):
            c = j0 // HALF
            off = j0 - c * HALF
            assert off + w <= HALF
            return xts[(b, c)][:, off:off + w]

        def scol(b, j0, w):
            c = j0 // HALF
            off = j0 - c * HALF
            assert off + w <= HALF
            return sqs[(b, c)][:, off:off + w]

        rmax2 = small.tile([P, BPC * MBLK, N // HALF], F32)
        # B/C paths only write slot 0 of a row; pre-fill with -inf-ish
        nc.gpsimd.memset(rmax2[:], -1.0e30)

        # main loop over (batch, row-block): two [128,1024] psum tiles per row
        for b in range(BPC):
            for m in range(MBLK):
                col = b * MBLK + m
                path = PATTERN[col % len(PATTERN)]
                lhs_x = xcol(b, m * P, P)
                phs = []
                for h in range(N // HALF):
                    ph = psum.tile([P, HALF], F32, tag="ph")
                    phs.append(ph)
                    diag_k = (m * P) // MMW
                    for k in range(HALF // MMW):
                        j0 = h * HALF + k * MMW
                        nc.tensor.matmul(
                            ph[:, k * MMW:(k + 1) * MMW],
                            lhs_x,
                            xcol(b, j0, MMW),
                            start=True, stop=False,
                        )
                    for k in range(HALF // MMW):
                        j0 = h * HALF + k * MMW
                        diag_here = diag_k == j0 // MMW
                        nc.tensor.matmul(
                            ph[:, k * MMW:(k + 1) * MMW],
                            neghalf[:],
                            scol(b, j0, MMW),
                            start=False, stop=not diag_here,
                        )
                        if diag_here:
                            off = m * P - h * HALF
                            nc.tensor.matmul(
                                ph[:, off:off + P],
                                identneg[:],
                                ident[:],
                                start=False, stop=True,
                            )

                if path == 'A':
                    for h in range(N // HALF):
                        nc.vector.tensor_reduce(
                            rmax2[:, col, h:h + 1], phs[h][:],
                            axis=mybir.AxisListType.X, op=mybir.AluOpType.max,
                        )
                else:
                    cps = []
                    for h in range(N // HALF):
                        cp = cpool.tile([P, HALF], BF16, tag="cp")
                        nc.scalar.copy(cp[:], phs[h][:])
                        cps.append(cp)
                    # 'B': bf16 TT-max folds at 2x, then a 512-wide reduce
                    t1 = fpool.tile([P, HALF], BF16, tag="bt1")
                    nc.vector.tensor_tensor(
                        t1[:], cps[0][:], cps[1][:], op=mybir.AluOpType.max
                    )
                    t2 = fpool.tile([P, HALF // 2], BF16, tag="bt2")
                    nc.vector.tensor_tensor(
                        t2[:], t1[:, :HALF // 2], t1[:, HALF // 2:],
                        op=mybir.AluOpType.max,
                    )
                    nc.vector.tensor_reduce(
                        rmax2[:, col, 0:1], t2[:],
                        axis=mybir.AxisListType.X, op=mybir.AluOpType.max,
                    )

        # sqn_i columns: tiny matmuls (emitted last; needed only at the end)
        psum_sqn = psq.tile([P, 2 * BPC * MBLK], F32)
        for b in range(BPC):
            for m in range(MBLK):
                col = b * MBLK + m
                nc.tensor.matmul(
                    psum_sqn[:, 2 * col:2 * col + 2],
                    scol(b, m * P, P),
                    onescol[:],
                    start=True, stop=True,
                )
        sqn_cols = small.tile([P, BPC * MBLK], F32)
        nc.vector.tensor_copy(
            sqn_cols[:],
            psum_sqn[:].rearrange("p (c t) -> p c t", t=2)[:, :, 0:1],
        )

        # nnd = sqrt(relu(sqn_i + (-2)*max))
        rmax = small.tile([P, BPC * MBLK], F32)
        nc.vector.tensor_reduce(
            rmax[:], rmax2[:], axis=mybir.AxisListType.X, op=mybir.AluOpType.max
        )
        nnd2 = small.tile([P, BPC * MBLK], F32)
        nc.vector.scalar_tensor_tensor(
            nnd2[:], rmax[:], -2.0, sqn_cols[:],
            op0=mybir.AluOpType.mult, op1=mybir.AluOpType.add,
        )
        nc.vector.tensor_scalar_max(nnd2[:], nnd2[:], 0.0)
        nnd_sb = small.tile([P, BPC * MBLK], F32)
        nc.scalar.sqrt(nnd_sb[:], nnd2[:])
        nc.sync.dma_start(nnd_d.ap()[:, :], nnd_sb[:])

    nc.compile()
    return nc


def _consts():
    neghalf = np.full((P, P), -0.5, dtype=ml_dtypes.bfloat16)
    onescol = np.ones((P, 2), dtype=ml_dtypes.bfloat16)
    identneg = (NEGBIG * np.eye(P)).astype(ml_dtypes.bfloat16)
    ident = np.eye(P, dtype=np.float32).astype(ml_dtypes.bfloat16)
    return neghalf, onescol, identneg, ident


def kernel(pred_poses: np.ndarray) -> np.ndarray:
    pred_poses = np.ascontiguousarray(np.asarray(pred_poses, dtype=np.float32))
    assert pred_poses.shape == (B, N, D)

    if "nc" not in _CACHE:
        _CACHE["nc"] = build_kernel()
    nc = _CACHE["nc"]

    neghalf, onescol, identneg, ident = _consts()
    in_maps = []
    for c in range(NCORES):
        xb = pred_poses[c * BPC:(c + 1) * BPC]          # [BPC, N, D]
        xt = np.ascontiguousarray(
            xb.transpose(0, 2, 1)).astype(ml_dtypes.bfloat16)  # [BPC, D, N]
        in_maps.append({
            "xt": xt,
            "neghalf": neghalf,
            "onescol": onescol,
            "identneg": identneg,
            "ident": ident,
        })

    res = run_bass_kernel_spmd(nc, in_maps, list(range(NCORES)))

    nnd = np.zeros((B, N), dtype=np.float64)
    for c in range(NCORES):
        t = np.asarray(res.results[c]["nnd"])           # [128, BPC*MBLK]
        for bl in range(BPC):
            sub = t[:, bl * MBLK:(bl + 1) * MBLK]       # [128, 16] (p, m)
            nnd[c * BPC + bl] = sub.T.reshape(N)        # index m*128+p

    mean = nnd.mean()
    std = nnd.std(ddof=1)
    eps = 1e-8
    cv = std / max(mean, eps) if mean > eps else 0.0
    return np.stack([mean, std, cv]).astype(np.float32)
